# revision 1
# baseline (speedup 1.0000x reference)
"""Deformable Transformer Encoder Layer — Trainium2 Bass kernel.

Sharding: 8 cores = (batch b in 0..3) x (query-half h in 0..1).
Each core computes the full layer for its (b, query-half) slice:
  - value projection over the FULL batch image (needed for sampling),
    written to DRAM scratch as bf16 (L, 256) row-major.
  - per 128-query block: projections (PE), sampling offsets + weights (DVE),
    gather via indirect DMA (64B runs: one pixel x one head's 32ch, bf16),
    blend (DVE mult + reduce), out-proj, LN, FFN, LN (PE/DVE/ACT).
No cross-core communication; host reassembles the output.
"""

import os
import sys
import numpy as np

for _p in ("/opt/trn_rl_repo", "/root/.axon_site/_ro/trn_rl_repo"):
    if os.path.isdir(_p) and _p not in sys.path:
        sys.path.insert(0, _p)

import concourse.bass as bass
import concourse.mybir as mybir
import concourse.tile as tile
from concourse import bacc
from concourse.bass import AP

F32 = mybir.dt.float32
BF16 = mybir.dt.bfloat16
I32 = mybir.dt.int32
I16 = mybir.dt.int16
AF = mybir.ActivationFunctionType
OP = mybir.AluOpType
AX = mybir.AxisListType

# Problem constants (checked against inputs at runtime on host).
M, LV, P, DM, DH, DF = 8, 4, 4, 256, 32, 1024
L = 11253
B = 4
LC = 5627           # queries per core (ceil(L/2) and floor alternate; we use split [5627, 5626])
LCPAD = 5632        # 44 * 128
NBLK = LCPAD // 128
RPAD = 11264        # value rows padded (88 * 128)
EPS = 1e-5
TWO23 = 12582912.0   # 3*2^22: x + (TWO23-0.5) lands in [2^23, 2^24) where ulp=1
DEBUG = False
NQUEUES = 1


def build_program():
    """Build the per-core Bass program. Same program for all 8 cores; the
    per-core differences are entirely in the input data."""
    nc = bacc.Bacc("TRN2", target_bir_lowering=False, debug=False, enable_asserts=False,
                   num_swdge_queues=NQUEUES)

    t = {}
    def inp(name, shape, dtype=F32):
        t[name] = nc.dram_tensor(name, list(shape), dtype, kind="ExternalInput").ap()
        return t[name]

    # per-core data
    inp("qf", (LCPAD, DM)); inp("qp", (LCPAD, DM)); inp("ref", (LCPAD, LV, 2))
    inp("srcb", (L, DM))
    # weights (bf16 on host for matmul rhs)
    inp("Woff", (DM, M * LV * P * 2), BF16)
    inp("Wattn", (DM, M * LV * P), BF16)
    inp("Wv", (DM, DM), BF16)
    inp("Wout", (DM, DM), BF16)
    inp("W1", (DM, DF), BF16)
    inp("W2", (DF, DM), BF16)
    # biases as [1, N] rows (rank-1 matmul trick), bf16
    inp("boff", (1, M * LV * P * 2), BF16)
    inp("battn", (1, M * LV * P), BF16)
    inp("bv", (1, DM), BF16)
    inp("bout", (1, DM), BF16)
    inp("b1", (1, DF), BF16)
    inp("b2", (1, DM), BF16)
    # layernorm params replicated across partitions (f32)
    inp("ln1g", (128, DM)); inp("ln1b", (128, DM))
    inp("ln2g", (128, DM)); inp("ln2b", (128, DM))
    # constants
    inp("ident", (128, 128))                  # f32 identity for PE transpose
    inp("ones1", (1, 128), BF16)              # rank-1 lhsT of ones
    inp("refdims", (128, LV * 2))             # (l,xy) -> W_l | H_l, replicated
    inp("dimm2", (128, M * LV * P * 2))       # (m,l,p,xy) -> dim-2, replicated
    inp("w8", (128, M * LV * P))              # (m,l,p) -> W_l * 8, replicated
    inp("c0", (128, M * LV * P))              # (m,l,p) -> lsi_l*8 + m, replicated

    out_ap = nc.dram_tensor("out", [LCPAD, DM], F32, kind="ExternalOutput").ap()
    if DEBUG:
        for nm in ("dbg_sampled", "dbg_attw", "dbg_x", "dbg_sw"):
            t[nm] = nc.dram_tensor(nm, [LCPAD, 512 if nm == "dbg_sw" else 256],
                                   F32, kind="ExternalOutput").ap()

    with tile.TileContext(nc) as tc:
        _build(tc, out_ap, t)

    nc.compile()
    return nc


def _build(tc, out_ap, t):
    nc = tc.nc
    from contextlib import ExitStack
    ctx = ExitStack()
    with ctx:
        consts = ctx.enter_context(tc.tile_pool(name="consts", bufs=1))
        wpool = ctx.enter_context(tc.tile_pool(name="wpool", bufs=1))
        vblk = ctx.enter_context(tc.tile_pool(name="vblk", bufs=3))
        blk = ctx.enter_context(tc.tile_pool(name="blk", bufs=2))
        big = ctx.enter_context(tc.tile_pool(name="big", bufs=2))
        big1 = ctx.enter_context(tc.tile_pool(name="big1", bufs=1))
        ps_t = ctx.enter_context(tc.tile_pool(name="ps_t", bufs=2, space="PSUM"))
        ps_mm = ctx.enter_context(tc.tile_pool(name="ps_mm", bufs=2, space="PSUM"))
        ps_f1 = ctx.enter_context(tc.tile_pool(name="ps_f1", bufs=2, space="PSUM"))
        dram = ctx.enter_context(tc.tile_pool(name="dram", bufs=1, space="DRAM"))

        # ---- resident constants / weights in SBUF ----
        def ld(name, shape=None, dt=None):
            ap = t[name]
            tile_ = consts.tile(list(ap.shape), ap.dtype, name=name + "_s")
            nc.sync.dma_start(out=tile_, in_=ap)
            return tile_

        ident = ld("ident")
        ones1 = ld("ones1")
        eps_t = consts.tile([128, 1], F32, name="eps_t")
        nc.vector.memset(eps_t, EPS)
        refdims = ld("refdims"); dimm2 = ld("dimm2"); w8 = ld("w8"); c0 = ld("c0")
        ln1g = ld("ln1g"); ln1b = ld("ln1b"); ln2g = ld("ln2g"); ln2b = ld("ln2b")
        boff = ld("boff"); battn = ld("battn"); bv = ld("bv")
        bout = ld("bout"); b1 = ld("b1"); b2 = ld("b2")

        def ldw(name, kchunks):
            """Load weight (K, N) as kchunks tiles of [128, N]."""
            ap = t[name]
            K, N = ap.shape
            tiles = []
            for k in range(kchunks):
                w_ = wpool.tile([128, N], ap.dtype, name=f"{name}_{k}")
                nc.sync.dma_start(out=w_, in_=ap[k * 128:(k + 1) * 128, :])
                tiles.append(w_)
            return tiles

        Woff = ldw("Woff", 2); Wattn = ldw("Wattn", 2); Wv = ldw("Wv", 2)
        Wout = ldw("Wout", 2); W1 = ldw("W1", 2); W2 = ldw("W2", 8)

        # ---- value projection -> DRAM scratch, pair-duplicated:
        # value2[r, m, 0:32] = v[r, m], value2[r, m, 32:64] = v[r+1, m]
        value2 = dram.tile([RPAD, M, 2 * DH], F32, name="value2")

        for vb in range((L + 127) // 128):
            p0 = vb * 128
            pn = min(128, L - p0)
            s_t = vblk.tile([128, DM], F32, name="s_t")
            nc.sync.dma_start(out=s_t[:pn], in_=t["srcb"][p0:p0 + pn, :])
            sT = []
            for c in range(2):
                tp = ps_t.tile([128, 128], F32, name="v_tp", tag="tp")
                nc.tensor.transpose(out=tp[:, :pn], in_=s_t[:pn, c * 128:(c + 1) * 128], identity=ident[:pn, :pn])
                sb = vblk.tile([128, 128], BF16, name="v_sT")
                nc.vector.tensor_copy(out=sb[:, :pn], in_=tp[:, :pn])
                sT.append(sb)
            pv = ps_mm.tile([128, DM], F32, name="v_ps", tag="mm")
            for c in range(2):
                nc.tensor.matmul(out=pv[:pn], lhsT=sT[c][:, :pn], rhs=Wv[c], start=(c == 0), stop=False)
            nc.tensor.matmul(out=pv[:pn], lhsT=ones1[:, :pn], rhs=bv, start=False, stop=True)
            vt = vblk.tile([128, DM], F32, name="v_out")
            nc.vector.tensor_copy(out=vt[:pn], in_=pv[:pn])
            vt_v = vt[:].rearrange("p (m c) -> p m c", c=DH)
            nc.sync.dma_start(out=value2[p0:p0 + pn, :, 0:DH], in_=vt_v[:pn])
            if vb == 0:
                nc.sync.dma_start(out=value2[0:pn - 1, :, DH:2 * DH], in_=vt_v[1:pn])
            else:
                nc.sync.dma_start(out=value2[p0 - 1:p0 + pn - 1, :, DH:2 * DH], in_=vt_v[:pn])

        # ---- main per-block loop ----
        for ib in range(NBLK):
            q0 = ib * 128
            qf_t = blk.tile([128, DM], F32, name="qf_t")
            qp_t = blk.tile([128, DM], F32, name="qp_t")
            ref_t = blk.tile([128, LV, 2], F32, name="ref_t")
            nc.sync.dma_start(out=qf_t, in_=t["qf"][q0:q0 + 128, :])
            nc.sync.dma_start(out=qp_t, in_=t["qp"][q0:q0 + 128, :])
            nc.sync.dma_start(out=ref_t, in_=t["ref"][q0:q0 + 128, :, :])

            query = blk.tile([128, DM], F32, name="query")
            nc.vector.tensor_tensor(out=query, in0=qf_t, in1=qp_t, op=OP.add)

            # transpose query -> qT bf16 chunks
            qT = []
            for c in range(2):
                tp = ps_t.tile([128, 128], F32, name="q_tp", tag="tp")
                nc.tensor.transpose(out=tp, in_=query[:, c * 128:(c + 1) * 128], identity=ident)
                sb = blk.tile([128, 128], BF16, name="qT")
                nc.vector.tensor_copy(out=sb, in_=tp)
                qT.append(sb)

            # offsets projection [128q, 256]
            ps_off = ps_mm.tile([128, 256], F32, name="ps_off", tag="mm")
            for c in range(2):
                nc.tensor.matmul(out=ps_off, lhsT=qT[c], rhs=Woff[c], start=(c == 0), stop=False)
            nc.tensor.matmul(out=ps_off, lhsT=ones1, rhs=boff, start=False, stop=True)

            # attention weights projection + softmax over (l,p) per head
            ps_at = ps_mm.tile([128, 128], F32, name="ps_at", tag="mm")
            for c in range(2):
                nc.tensor.matmul(out=ps_at, lhsT=qT[c], rhs=Wattn[c], start=(c == 0), stop=False)
            nc.tensor.matmul(out=ps_at, lhsT=ones1, rhs=battn, start=False, stop=True)
            expt = blk.tile([128, 128], F32, name="expt")
            nc.scalar.activation(out=expt, in_=ps_at, func=AF.Exp)
            den = blk.tile([128, M], F32, name="den")
            nc.vector.tensor_reduce(out=den, in_=expt[:].rearrange("p (m k) -> p m k", m=M),
                                    axis=AX.X, op=OP.add)
            nc.vector.reciprocal(out=den, in_=den)
            attw = blk.tile([128, 128], F32, name="attw")
            nc.vector.tensor_tensor(out=attw[:].rearrange("p (m k) -> p m k", m=M),
                                    in0=expt[:].rearrange("p (m k) -> p m k", m=M),
                                    in1=den[:, :, None].broadcast_to([128, M, LV * P]),
                                    op=OP.mult)

            # ---- sampling math (all [128, 256] tiles over (m,l,p,xy)) ----
            refx = blk.tile([128, LV * 2], F32, name="refx")
            nc.vector.tensor_tensor(out=refx, in0=ref_t[:].rearrange("p l x -> p (l x)"),
                                    in1=refdims, op=OP.mult)
            nc.vector.tensor_scalar(out=refx, in0=refx, scalar1=0.5, scalar2=None, op0=OP.subtract)

            # expand refx (l,xy) -> (l,p,xy) then broadcast over m in the add
            refx32 = blk.tile([128, LV * P * 2], F32, name="refx32")
            nc.vector.tensor_copy(
                out=refx32[:].rearrange("p (l q x) -> p l q x", l=LV, q=P),
                in_=refx[:].rearrange("p (l x) -> p l x", x=2)[:, :, None, :]
                    .broadcast_to([128, LV, P, 2]))
            x = blk.tile([128, 256], F32, name="x")
            nc.vector.tensor_tensor(
                out=x[:].rearrange("p (m k) -> p m k", m=M),
                in0=ps_off[:].rearrange("p (m k) -> p m k", m=M),
                in1=refx32[:, None, :].broadcast_to([128, M, LV * P * 2]),
                op=OP.add)
            x0 = blk.tile([128, 256], F32, name="x0")
            # floor via round(x - 0.5): ((x - 0.5) + C) - C with C = 3*2^22
            # (C - 0.5 itself is not fp32-representable, so two instructions)
            nc.vector.tensor_scalar(out=x0, in0=x, scalar1=0.5, scalar2=TWO23,
                                    op0=OP.subtract, op1=OP.add)
            nc.vector.tensor_scalar(out=x0, in0=x0, scalar1=TWO23, scalar2=None,
                                    op0=OP.subtract)
            w = blk.tile([128, 256], F32, name="w")
            nc.vector.tensor_tensor(out=w, in0=x, in1=x0, op=OP.subtract)
            base = blk.tile([128, 256], F32, name="base")
            nc.vector.tensor_scalar(out=base, in0=x0, scalar1=0.0, scalar2=None, op0=OP.max)
            nc.vector.tensor_tensor(out=base, in0=base, in1=dimm2, op=OP.min)
            d = blk.tile([128, 256], F32, name="d")
            nc.vector.tensor_tensor(out=d, in0=x0, in1=base, op=OP.subtract)
            i0 = blk.tile([128, 256], F32, name="i0")
            im = blk.tile([128, 256], F32, name="im")
            ip = blk.tile([128, 256], F32, name="ip")
            nc.vector.tensor_scalar(out=i0, in0=d, scalar1=0.0, scalar2=None, op0=OP.is_equal)
            nc.vector.tensor_scalar(out=im, in0=d, scalar1=-1.0, scalar2=None, op0=OP.is_equal)
            nc.vector.tensor_scalar(out=ip, in0=d, scalar1=1.0, scalar2=None, op0=OP.is_equal)
            wi0 = blk.tile([128, 256], F32, name="wi0")
            nc.vector.tensor_tensor(out=wi0, in0=w, in1=i0, op=OP.mult)
            s0 = blk.tile([128, 256], F32, name="s0")
            s1 = blk.tile([128, 256], F32, name="s1")
            # s0 = i0 - w*i0 + w*im ; s1 = (ip - w*ip) + w*i0
            wim = blk.tile([128, 256], F32, name="wim")
            nc.vector.tensor_tensor(out=wim, in0=w, in1=im, op=OP.mult)
            nc.vector.tensor_tensor(out=s0, in0=i0, in1=wi0, op=OP.subtract)
            nc.vector.tensor_tensor(out=s0, in0=s0, in1=wim, op=OP.add)
            wip = blk.tile([128, 256], F32, name="wip")
            nc.vector.tensor_tensor(out=wip, in0=w, in1=ip, op=OP.mult)
            nc.vector.tensor_tensor(out=s1, in0=ip, in1=wip, op=OP.subtract)
            nc.vector.tensor_tensor(out=s1, in0=s1, in1=wi0, op=OP.add)

            # ---- gather row indices (per head, row = lsi_l + basey*W_l + basex) ----
            # base even entries = basex, odd = basey (per (m,l,p))
            b_ev = base[:].rearrange("p (k x) -> p k x", x=2)[:, :, 0]
            b_od = base[:].rearrange("p (k x) -> p k x", x=2)[:, :, 1]
            y0off = blk.tile([128, 128], F32, name="y0off")
            nc.vector.tensor_tensor(out=y0off, in0=b_od, in1=w8, op=OP.mult)
            nc.vector.tensor_tensor(out=y0off, in0=y0off, in1=c0, op=OP.add)
            nc.vector.tensor_tensor(out=y0off, in0=y0off, in1=b_ev, op=OP.add)
            y1off = blk.tile([128, 128], F32, name="y1off")
            nc.vector.tensor_tensor(out=y1off, in0=y0off, in1=w8, op=OP.add)

            # staging int16 [128, (m,l,p,y)]
            offs16 = blk.tile([128, 256], I16, name="offs16")
            nc.vector.memset(offs16, 0)
            o16 = offs16[:].rearrange("p (k y) -> p k y", y=2)
            nc.vector.tensor_copy(out=o16[:, :, 0], in_=y0off)
            nc.vector.tensor_copy(out=o16[:, :, 1], in_=y1off)

            # fold partitions (q = qhi*16+qlo) -> [16 qlo, qhi 8, (m,c) 256]
            fold1 = blk.tile([16, 8, 256], I16, name="fold1")
            for qhi in range(8):
                nc.sync.dma_start(
                    out=fold1[:, qhi, :],
                    in_=offs16[qhi * 16:(qhi + 1) * 16, :])

            # free-dim transpose (qhi, m, c) -> (m, c, qhi) on DVE (16 lanes),
            # written into rows 0..15 of the replicated idx tile
            fold2r = blk.tile([128, M * 256], I16, name="fold2r")
            nc.vector.tensor_copy(
                out=fold2r[0:16, :].rearrange("p (m c q) -> p m q c", m=M, c=32, q=8),
                in_=fold1[:].rearrange("p q (m c) -> p m q c", m=M, c=32))
            # replicate rows 0..15 to all 128 partitions (wrapped idx layout
            # must be identical in every 16-partition group)
            nc.sync.dma_start(out=fold2r[16:32, :], in_=fold2r[0:16, :])
            nc.sync.dma_start(out=fold2r[32:64, :], in_=fold2r[0:32, :])
            nc.sync.dma_start(out=fold2r[64:128, :], in_=fold2r[0:64, :])

            # ---- combined sample weights SW bf16 [128, (m,l,p) 128, (y,s) 4] ----
            s0_ev = s0[:].rearrange("p (k x) -> p k x", x=2)[:, :, 0]
            s0_od = s0[:].rearrange("p (k x) -> p k x", x=2)[:, :, 1]
            s1_ev = s1[:].rearrange("p (k x) -> p k x", x=2)[:, :, 0]
            s1_od = s1[:].rearrange("p (k x) -> p k x", x=2)[:, :, 1]
            tmp0 = blk.tile([128, 128], F32, name="tmp0")
            tmp1 = blk.tile([128, 128], F32, name="tmp1")
            nc.vector.tensor_tensor(out=tmp0, in0=attw, in1=s0_od, op=OP.mult)
            nc.vector.tensor_tensor(out=tmp1, in0=attw, in1=s1_od, op=OP.mult)
            sw = blk.tile([128, 512], F32, name="sw")
            swv = sw[:].rearrange("p (k u) -> p k u", u=4)
            nc.vector.tensor_tensor(out=swv[:, :, 0], in0=tmp0, in1=s0_ev, op=OP.mult)
            nc.vector.tensor_tensor(out=swv[:, :, 1], in0=tmp0, in1=s1_ev, op=OP.mult)
            nc.vector.tensor_tensor(out=swv[:, :, 2], in0=tmp1, in1=s0_ev, op=OP.mult)
            nc.vector.tensor_tensor(out=swv[:, :, 3], in0=tmp1, in1=s1_ev, op=OP.mult)

            # ---- gather + blend per head ----
            # g2m[q, c, s*32+ch] = value2[row(q,c), m, :]; row pairs (x0, x0+1)
            sampled = blk.tile([128, DM], F32, name="sampled")
            for m in range(M):
                g, mm = divmod(m, 4)
                g2m = big.tile([128, 32, 2 * DH], F32, name="g2m")
                for k in range(4):
                    nc.gpsimd.dma_gather(
                        out_ap=g2m[:, k * 8:(k + 1) * 8, :],
                        in_ap=value2[:, m, :],
                        idxs_ap=fold2r[:, m * 256 + k * 64: m * 256 + (k + 1) * 64],
                        num_idxs=1024, num_idxs_reg=1024,
                        elem_size=2 * DH, elem_step=M * 2 * DH,
                        transpose=False, queue_num=mm % NQUEUES)
                # weighted (layout ch-major for unit-stride reduce)
                wtm = big1.tile([128, DH * 64], F32, name="wtm")
                swm = sw[:, m * 64:(m + 1) * 64].rearrange("p (c s) -> p c s", s=2)
                nc.vector.tensor_tensor(
                    out=wtm[:].rearrange("p (ch c s) -> p c s ch", ch=DH, c=32, s=2),
                    in0=g2m[:].rearrange("p c (s ch) -> p c s ch", s=2),
                    in1=swm[:, :, :, None].broadcast_to([128, 32, 2, DH]),
                    op=OP.mult)
                nc.vector.tensor_reduce(
                    out=sampled[:, m * DH:(m + 1) * DH],
                    in_=wtm[:].rearrange("p (ch u) -> p ch u", ch=DH),
                    axis=AX.X, op=OP.add)

            if DEBUG:
                nc.sync.dma_start(out=t["dbg_sampled"][q0:q0 + 128, :], in_=sampled)
                nc.sync.dma_start(out=t["dbg_attw"][q0:q0 + 128, 0:128], in_=attw)
                nc.sync.dma_start(out=t["dbg_x"][q0:q0 + 128, :], in_=x)
                dbg_sw_t = blk.tile([128, 512], F32, name="dbg_sw_t")
                nc.vector.tensor_copy(out=dbg_sw_t, in_=sw)
                nc.sync.dma_start(out=t["dbg_sw"][q0:q0 + 128, :], in_=dbg_sw_t)

            # ---- output projection ----
            sT = []
            for c in range(2):
                tp = ps_t.tile([128, 128], F32, name="s_tp", tag="tp")
                nc.tensor.transpose(out=tp, in_=sampled[:, c * 128:(c + 1) * 128], identity=ident)
                sb = blk.tile([128, 128], BF16, name="sT")
                nc.vector.tensor_copy(out=sb, in_=tp)
                sT.append(sb)
            ps_h = ps_mm.tile([128, DM], F32, name="ps_h", tag="mm")
            for c in range(2):
                nc.tensor.matmul(out=ps_h, lhsT=sT[c], rhs=Wout[c], start=(c == 0), stop=False)
            nc.tensor.matmul(out=ps_h, lhsT=ones1, rhs=bout, start=False, stop=True)

            # ---- LN1 ----
            r1 = blk.tile([128, DM], F32, name="r1")
            nc.vector.tensor_tensor(out=r1, in0=qf_t, in1=ps_h, op=OP.add)
            h = _layernorm(nc, blk, r1, ln1g, ln1b, eps_t)

            # ---- FFN ----
            hT = []
            for c in range(2):
                tp = ps_t.tile([128, 128], F32, name="h_tp", tag="tp")
                nc.tensor.transpose(out=tp, in_=h[:, c * 128:(c + 1) * 128], identity=ident)
                sb = blk.tile([128, 128], BF16, name="hT")
                nc.vector.tensor_copy(out=sb, in_=tp)
                hT.append(sb)
            relu1 = []
            for fc in range(8):
                pf = ps_f1.tile([128, 128], F32, name="pf")
                for c in range(2):
                    nc.tensor.matmul(out=pf, lhsT=W1[c][:, fc * 128:(fc + 1) * 128],
                                     rhs=hT[c], start=(c == 0), stop=False)
                nc.tensor.matmul(out=pf, lhsT=b1[:, fc * 128:(fc + 1) * 128],
                                 rhs=ones1, start=False, stop=True)
                rt = blk.tile([128, 128], BF16, name=f"relu1_{fc}")
                nc.scalar.activation(out=rt, in_=pf, func=AF.Relu)
                relu1.append(rt)
            ps_o = ps_mm.tile([128, DM], F32, name="ps_o", tag="mm")
            for fc in range(8):
                nc.tensor.matmul(out=ps_o, lhsT=relu1[fc], rhs=W2[fc], start=(fc == 0), stop=False)
            nc.tensor.matmul(out=ps_o, lhsT=ones1, rhs=b2, start=False, stop=True)

            # ---- LN2 + store ----
            r2 = blk.tile([128, DM], F32, name="r2")
            nc.vector.tensor_tensor(out=r2, in0=h, in1=ps_o, op=OP.add)
            o = _layernorm(nc, blk, r2, ln2g, ln2b, eps_t)
            nc.sync.dma_start(out=out_ap[q0:q0 + 128, :], in_=o)


def _layernorm(nc, pool, r, g, b, eps_t):
    stats = pool.tile([128, 6], F32, name="ln_stats")
    nc.vector.bn_stats(out=stats, in_=r)
    mv = pool.tile([128, 2], F32, name="ln_mv")
    nc.vector.bn_aggr(out=mv, in_=stats)
    rstd = pool.tile([128, 1], F32, name="ln_rstd")
    nc.scalar.activation(out=rstd, in_=mv[:, 1:2], func=AF.Sqrt, bias=eps_t)
    nc.vector.reciprocal(out=rstd, in_=rstd)
    xs = pool.tile([128, DM], F32, name="ln_xs")
    nc.vector.tensor_scalar(out=xs, in0=r, scalar1=mv[:, 0:1], scalar2=rstd,
                            op0=OP.subtract, op1=OP.mult)
    h = pool.tile([128, DM], F32, name="ln_h")
    nc.vector.tensor_tensor(out=h, in0=xs, in1=g, op=OP.mult)
    nc.vector.tensor_tensor(out=h, in0=h, in1=b, op=OP.add)
    return h


# ---------------------------------------------------------------------------
# host side
# ---------------------------------------------------------------------------

_prog_cache = {}


def _get_program():
    if "nc" not in _prog_cache:
        _prog_cache["nc"] = build_program()
    return _prog_cache["nc"]


def _host_constants(ss, lsi):
    """Input-independent constant tiles derived from spatial shapes."""
    ss = np.asarray(ss, np.int64)
    lsi = np.asarray(lsi, np.int64)
    f = np.float32
    H = ss[:, 0].astype(np.int64)
    W = ss[:, 1].astype(np.int64)
    # refdims [(l, xy)] : xy=0 -> W, xy=1 -> H
    refd = np.zeros((LV, 2), f)
    refd[:, 0] = W; refd[:, 1] = H
    refdims = np.broadcast_to(refd.reshape(1, -1), (128, LV * 2)).copy()
    # dimm2 [(m,l,p,xy)] -> dim - 2
    dm2 = np.zeros((M, LV, P, 2), f)
    dm2[:, :, :, 0] = (W - 2)[None, :, None]
    dm2[:, :, :, 1] = (H - 2)[None, :, None]
    dimm2 = np.broadcast_to(dm2.reshape(1, -1), (128, M * LV * P * 2)).copy()
    # w8 [(m,l,p)] -> W_l (row stride within level)
    w8a = np.zeros((M, LV, P), f)
    w8a[:, :, :] = W[None, :, None]
    w8 = np.broadcast_to(w8a.reshape(1, -1), (128, M * LV * P)).copy()
    # c0 [(m,l,p)] -> lsi_l
    c0a = np.zeros((M, LV, P), f)
    c0a[:, :, :] = lsi[None, :, None]
    c0 = np.broadcast_to(c0a.reshape(1, -1), (128, M * LV * P)).copy()
    return refdims, dimm2, w8, c0


def _build_in_maps(inputs):
    src = np.asarray(inputs["src"], np.float32)
    q_feat = np.asarray(inputs["q_feat"], np.float32)
    q_pos = np.asarray(inputs["q_pos"], np.float32)
    ref = np.asarray(inputs["reference_points"], np.float32)
    ss = np.asarray(inputs["spatial_shapes"])
    lsi_in = np.asarray(inputs["level_start_index"])

    lsi = lsi_in.astype(np.int64)

    assert src.shape == (B, L, DM), src.shape
    refdims, dimm2, w8, c0 = _host_constants(ss, lsi)

    def as_bf16(a):
        import ml_dtypes
        return np.asarray(a, np.float32).astype(ml_dtypes.bfloat16)

    common = {
        "Woff": as_bf16(inputs["W_off"]),
        "Wattn": as_bf16(inputs["W_attn"]),
        "Wv": as_bf16(inputs["W_v"]),
        "Wout": as_bf16(inputs["W_out"]),
        "W1": as_bf16(inputs["W1"]),
        "W2": as_bf16(inputs["W2"]),
        "boff": as_bf16(inputs["b_off"]).reshape(1, -1),
        "battn": as_bf16(inputs["b_attn"]).reshape(1, -1),
        "bv": as_bf16(inputs["b_v"]).reshape(1, -1),
        "bout": as_bf16(inputs["b_out"]).reshape(1, -1),
        "b1": as_bf16(inputs["b1"]).reshape(1, -1),
        "b2": as_bf16(inputs["b2"]).reshape(1, -1),
        "ln1g": np.broadcast_to(np.asarray(inputs["ln1_g"], np.float32), (128, DM)).copy(),
        "ln1b": np.broadcast_to(np.asarray(inputs["ln1_b"], np.float32), (128, DM)).copy(),
        "ln2g": np.broadcast_to(np.asarray(inputs["ln2_g"], np.float32), (128, DM)).copy(),
        "ln2b": np.broadcast_to(np.asarray(inputs["ln2_b"], np.float32), (128, DM)).copy(),
        "ident": np.eye(128, dtype=np.float32),
        "ones1": as_bf16(np.ones((1, 128), np.float32)),
        "refdims": refdims, "dimm2": dimm2, "w8": w8, "c0": c0,
    }

    halves = [(0, LC), (LC, L - LC)]  # (start, count) per half
    in_maps = []
    for core in range(8):
        b = core // 2
        h0, hn = halves[core % 2]
        qf = np.zeros((LCPAD, DM), np.float32)
        qp = np.zeros((LCPAD, DM), np.float32)
        rf = np.zeros((LCPAD, LV, 2), np.float32)
        qf[:hn] = q_feat[b, h0:h0 + hn]
        qp[:hn] = q_pos[b, h0:h0 + hn]
        rf[:hn] = ref[b, h0:h0 + hn]
        m = dict(common)
        m.update({"qf": qf, "qp": qp, "ref": rf, "srcb": src[b]})
        in_maps.append(m)
    return in_maps


def kernel(**inputs):
    from concourse.bass_utils import run_bass_kernel_spmd

    in_maps = _build_in_maps(inputs)
    nc = _get_program()
    res = run_bass_kernel_spmd(nc, in_maps, core_ids=list(range(8)))

    halves = [(0, LC), (LC, L - LC)]
    out = np.zeros((B, L, DM), np.float32)
    for core in range(8):
        b = core // 2
        h0, hn = halves[core % 2]
        out[b, h0:h0 + hn] = res.results[core]["out"][:hn]
    return out



# revision 16
# speedup vs baseline: 1.9112x; 1.9112x over previous
"""Deformable Transformer Encoder Layer — Trainium2 Bass kernel (v2).

Sharding: 8 cores = (batch b in 0..3) x (query-half h in 0..1).
Each core computes the full layer for its (b, query-half) slice.

v2 changes vs v1:
  - value stored bf16, quad-duplicated: value4[r, m, 0:128] =
    [v[r], v[r+1], v[r+W_l], v[r+W_l+1]] (W_l = level width of row r's
    level), so one 256B gather fetch covers a full 2x2 bilinear patch.
    One dma_gather per (block, head): 2048 idxs (vs 4x1024 per head
    with 2 fetches/point in v1) -> half the DMA descriptors, 1/4 the
    SWDGE calls.
  - bilinear edge weights via s0 = -min(|x-base|-1, 0),
    s1 = -min(|x-base-1|-1, 0) (negations cancel in products):
    8 DVE ops/block instead of 17.
  - blend in bf16 with pair-duplicated weights (innermost stride-1
    pair) so the big multiply runs in DVE 2x mode; reduction is a bf16
    add-tree + small f32 tensor_reduce.
"""

import os
import sys
import numpy as np

for _p in ("/opt/trn_rl_repo", "/root/.axon_site/_ro/trn_rl_repo"):
    if os.path.isdir(_p) and _p not in sys.path:
        sys.path.insert(0, _p)

import concourse.bass as bass
import concourse.mybir as mybir
import concourse.tile as tile
from concourse import bacc
from concourse.bass import AP

F32 = mybir.dt.float32
BF16 = mybir.dt.bfloat16
I32 = mybir.dt.int32
I16 = mybir.dt.int16
AF = mybir.ActivationFunctionType
OP = mybir.AluOpType
AX = mybir.AxisListType

# Problem constants (checked against inputs at runtime on host).
M, LV, P, DM, DH, DF = 8, 4, 4, 256, 32, 1024
NPT = LV * P        # 16 sample points per (query, head)
L = 11253
B = 4
LC = 5627           # queries per core (split [5627, 5626])
LCPAD = 5632        # 44 * 128
NBLK = LCPAD // 128
RPAD = 11264        # value rows padded (88 * 128)
EPS = 1e-5
TWO23 = 12582912.0   # 3*2^22: x + (TWO23-0.5) lands in [2^23, 2^24) where ulp=1
SHAPES = [(92, 92), (46, 46), (23, 23), (12, 12)]
LSI = [0, 8464, 10580, 11109]
NQUEUES = 1


def build_program():
    nc = bacc.Bacc("TRN2", target_bir_lowering=False, debug=False, enable_asserts=False,
                   num_swdge_queues=NQUEUES)

    t = {}
    def inp(name, shape, dtype=F32):
        t[name] = nc.dram_tensor(name, list(shape), dtype, kind="ExternalInput").ap()
        return t[name]

    # per-core data
    inp("qf", (LCPAD, DM)); inp("qp", (LCPAD, DM)); inp("ref", (LCPAD, LV, 2))
    inp("srcb", (L, DM))
    # weights (bf16 on host for matmul rhs)
    inp("Woff", (DM, M * NPT * 2), BF16)
    inp("Wattn", (DM, M * NPT), BF16)
    inp("Wv", (DM, DM), BF16)
    inp("Wout", (DM, DM), BF16)
    inp("W1", (DM, DF), BF16)
    inp("W2", (DF, DM), BF16)
    # biases as [1, N] rows (rank-1 matmul trick), bf16
    inp("boff", (1, M * NPT * 2), BF16)
    inp("battn", (1, M * NPT), BF16)
    inp("bv", (1, DM), BF16)
    inp("bout", (1, DM), BF16)
    inp("b1", (1, DF), BF16)
    inp("b2", (1, DM), BF16)
    # layernorm params replicated across partitions (f32)
    inp("ln1g", (128, DM)); inp("ln1b", (128, DM))
    inp("ln2g", (128, DM)); inp("ln2b", (128, DM))
    # constants
    inp("ident", (128, 128), BF16)            # bf16 identity for PE transpose
    inp("identf", (128, 128))                 # f32 identity for PE transpose
    inp("ones1", (1, 128), BF16)              # rank-1 lhsT of ones
    inp("refdims", (128, LV * 2))             # (l,xy) -> W_l | H_l, replicated
    inp("dimm2", (128, M * NPT * 2))          # (m,l,p,xy) -> dim-2, replicated
    inp("w8", (128, M * NPT))                 # (m,l,p) -> W_l, replicated
    inp("c0", (128, M * NPT))                 # (m,l,p) -> lsi_l, replicated

    out_ap = nc.dram_tensor("out", [LCPAD, DM], F32, kind="ExternalOutput").ap()

    with tile.TileContext(nc) as tc:
        _build(tc, out_ap, t)

    nc.compile()
    return nc


def _build(tc, out_ap, t):
    nc = tc.nc
    from contextlib import ExitStack
    ctx = ExitStack()
    with ctx:
        consts = ctx.enter_context(tc.tile_pool(name="consts", bufs=1))
        wpool = ctx.enter_context(tc.tile_pool(name="wpool", bufs=1))
        vblk = ctx.enter_context(tc.tile_pool(name="vblk", bufs=3))
        blk = ctx.enter_context(tc.tile_pool(name="blk", bufs=2))
        big = ctx.enter_context(tc.tile_pool(name="big", bufs=3))
        ps_t = ctx.enter_context(tc.tile_pool(name="ps_t", bufs=2, space="PSUM"))
        ps_mm = ctx.enter_context(tc.tile_pool(name="ps_mm", bufs=2, space="PSUM"))
        ps_f1 = ctx.enter_context(tc.tile_pool(name="ps_f1", bufs=2, space="PSUM"))
        dram = ctx.enter_context(tc.tile_pool(name="dram", bufs=1, space="DRAM"))

        # ---- resident constants / weights in SBUF ----
        def ld(name):
            ap = t[name]
            tile_ = consts.tile(list(ap.shape), ap.dtype, name=name + "_s")
            nc.sync.dma_start(out=tile_, in_=ap)
            return tile_

        ident = ld("ident")
        identf = ld("identf")
        ones1 = ld("ones1")
        eps_t = consts.tile([128, 1], F32, name="eps_t")
        nc.vector.memset(eps_t, EPS)
        refdims = ld("refdims"); dimm2 = ld("dimm2"); w8 = ld("w8"); c0 = ld("c0")
        ln1g = ld("ln1g"); ln1b = ld("ln1b"); ln2g = ld("ln2g"); ln2b = ld("ln2b")
        boff = ld("boff"); battn = ld("battn"); bv = ld("bv")
        bout = ld("bout"); b1 = ld("b1"); b2 = ld("b2")

        def ldw(name, kchunks):
            ap = t[name]
            K, N = ap.shape
            tiles = []
            for k in range(kchunks):
                w_ = wpool.tile([128, N], ap.dtype, name=f"{name}_{k}")
                nc.sync.dma_start(out=w_, in_=ap[k * 128:(k + 1) * 128, :])
                tiles.append(w_)
            return tiles

        Woff = ldw("Woff", 2); Wattn = ldw("Wattn", 2); Wv = ldw("Wv", 2)
        Wout = ldw("Wout", 2); W1 = ldw("W1", 2); W2 = ldw("W2", 8)

        # ---- value projection -> DRAM scratch, quad-duplicated bf16:
        # value4[r, m, 0:32]=v[r,m]  [32:64]=v[r+1,m]
        #            [64:96]=v[r+W_l,m]  [96:128]=v[r+W_l+1,m]
        # (W_l = width of the level containing dst row r)
        value4 = dram.tile([RPAD, M, 4 * DH], BF16, name="value4")

        for vb in range((L + 127) // 128):
            p0 = vb * 128
            pn = min(128, L - p0)
            s_t = vblk.tile([128, DM], F32, name="s_t")
            nc.sync.dma_start(out=s_t[:pn], in_=t["srcb"][p0:p0 + pn, :])
            sT = []
            for c in range(2):
                tp = ps_t.tile([128, 128], F32, name="v_tp", tag="tp")
                nc.tensor.transpose(out=tp[:, :pn], in_=s_t[:pn, c * 128:(c + 1) * 128],
                                    identity=identf[:pn, :pn])
                sb = vblk.tile([128, 128], BF16, name="v_sT")
                nc.vector.tensor_copy(out=sb[:, :pn], in_=tp[:, :pn])
                sT.append(sb)
            pv = ps_mm.tile([128, DM], F32, name="v_ps", tag="mm")
            for c in range(2):
                nc.tensor.matmul(out=pv[:pn], lhsT=sT[c][:, :pn], rhs=Wv[c], start=(c == 0), stop=False)
            nc.tensor.matmul(out=pv[:pn], lhsT=ones1[:, :pn], rhs=bv, start=False, stop=True)
            vt = vblk.tile([128, DM], BF16, name="v_out")
            nc.vector.tensor_copy(out=vt[:pn], in_=pv[:pn])
            vt_v = vt[:].rearrange("p (m c) -> p m c", c=DH)
            # px0: value4[r,:,0:32] = v[r]
            nc.sync.dma_start(out=value4[p0:p0 + pn, :, 0:DH], in_=vt_v[:pn])
            # px1: value4[r-1,:,32:64] = v[r]
            if vb == 0:
                nc.scalar.dma_start(out=value4[0:pn - 1, :, DH:2 * DH], in_=vt_v[1:pn])
            else:
                nc.scalar.dma_start(out=value4[p0 - 1:p0 + pn - 1, :, DH:2 * DH], in_=vt_v[:pn])
            # px2/px3: value4[r-W,:,64:96] = v[r]; value4[r-W-1,:,96:128] = v[r]
            # (level-dependent W; a 128-row src block can straddle one level
            #  boundary -> up to 2 pieces per shift)
            for li, (H, W) in enumerate(SHAPES):
                lvl0 = LSI[li]
                lvl1 = LSI[li] + H * W
                for shift, o0 in ((W, 2 * DH), (W + 1, 3 * DH)):
                    # dst rows r (in level li) receiving src rows r+shift
                    # from this block: r in [lvl0, lvl1) & [p0-shift, p0+pn-shift)
                    d0 = max(lvl0, p0 - shift)
                    d1 = min(lvl1, p0 + pn - shift)
                    if d0 >= d1:
                        continue
                    s0_, s1_ = d0 + shift - p0, d1 + shift - p0
                    nc.scalar.dma_start(out=value4[d0:d1, :, o0:o0 + DH],
                                        in_=vt_v[s0_:s1_])

        # ---- main per-block loop, software-pipelined emission:
        # front(n+1) is emitted before back(n) so each engine's in-order
        # queue interleaves the independent stages of adjacent blocks
        # (avoids head-of-line blocking on PE/SP sequencers).
        def front(ib):
            q0 = ib * 128
            qf_t = blk.tile([128, DM], F32, name="qf_t")
            qp_t = blk.tile([128, DM], F32, name="qp_t")
            ref_t = blk.tile([128, LV, 2], F32, name="ref_t")
            nc.sync.dma_start(out=qf_t, in_=t["qf"][q0:q0 + 128, :])
            nc.sync.dma_start(out=qp_t, in_=t["qp"][q0:q0 + 128, :])
            nc.sync.dma_start(out=ref_t, in_=t["ref"][q0:q0 + 128, :, :])

            query = blk.tile([128, DM], F32, name="query")
            nc.vector.tensor_tensor(out=query, in0=qf_t, in1=qp_t, op=OP.add)

            # transpose query -> qT bf16 chunks
            qT = []
            for c in range(2):
                tp = ps_t.tile([128, 128], F32, name="q_tp", tag="tp")
                nc.tensor.transpose(out=tp, in_=query[:, c * 128:(c + 1) * 128], identity=identf)
                sb = blk.tile([128, 128], BF16, name="qT")
                nc.vector.tensor_copy(out=sb, in_=tp)
                qT.append(sb)

            # offsets projection [128q, 256]
            ps_off = ps_mm.tile([128, 256], F32, name="ps_off", tag="mm")
            for c in range(2):
                nc.tensor.matmul(out=ps_off, lhsT=qT[c], rhs=Woff[c], start=(c == 0), stop=False)
            nc.tensor.matmul(out=ps_off, lhsT=ones1, rhs=boff, start=False, stop=True)

            # attention weights projection + softmax over (l,p) per head
            ps_at = ps_mm.tile([128, 128], F32, name="ps_at", tag="mm")
            for c in range(2):
                nc.tensor.matmul(out=ps_at, lhsT=qT[c], rhs=Wattn[c], start=(c == 0), stop=False)
            nc.tensor.matmul(out=ps_at, lhsT=ones1, rhs=battn, start=False, stop=True)
            expt = blk.tile([128, 128], F32, name="expt")
            nc.scalar.activation(out=expt, in_=ps_at, func=AF.Exp)
            den = blk.tile([128, M], F32, name="den")
            nc.vector.tensor_reduce(out=den, in_=expt[:].rearrange("p (m k) -> p m k", m=M),
                                    axis=AX.X, op=OP.add)
            nc.vector.reciprocal(out=den, in_=den)
            attw = blk.tile([128, 128], F32, name="attw")
            nc.vector.tensor_tensor(out=attw[:].rearrange("p (m k) -> p m k", m=M),
                                    in0=expt[:].rearrange("p (m k) -> p m k", m=M),
                                    in1=den[:, :, None].broadcast_to([128, M, NPT]),
                                    op=OP.mult)

            # ---- sampling math (all [128, 256] tiles over (m,l,p,xy)) ----
            # x = ref*dims - 0.5 + off   (offset normalizer cancels: off/W_norm*W = off)
            refx = blk.tile([128, LV * 2], F32, name="refx")
            nc.vector.tensor_tensor(out=refx, in0=ref_t[:].rearrange("p l x -> p (l x)"),
                                    in1=refdims, op=OP.mult)
            nc.vector.tensor_scalar(out=refx, in0=refx, scalar1=0.5, scalar2=None, op0=OP.subtract)
            refx32 = blk.tile([128, LV * P * 2], F32, name="refx32")
            nc.vector.tensor_copy(
                out=refx32[:].rearrange("p (l q x) -> p l q x", l=LV, q=P),
                in_=refx[:].rearrange("p (l x) -> p l x", x=2)[:, :, None, :]
                    .broadcast_to([128, LV, P, 2]))
            x = blk.tile([128, 256], F32, name="x")
            nc.vector.tensor_tensor(
                out=x[:].rearrange("p (m k) -> p m k", m=M),
                in0=ps_off[:].rearrange("p (m k) -> p m k", m=M),
                in1=refx32[:, None, :].broadcast_to([128, M, NPT * 2]),
                op=OP.add)
            # base = clamp(floor(x), 0, dim-2); t = x - base
            # floor via round(x-0.5): ((x-0.5)+C)-C with C=3*2^22 (fuse max-0)
            x0 = blk.tile([128, 256], F32, name="x0")
            nc.vector.tensor_scalar(out=x0, in0=x, scalar1=0.5, scalar2=TWO23,
                                    op0=OP.subtract, op1=OP.add)
            nc.vector.tensor_scalar(out=x0, in0=x0, scalar1=TWO23, scalar2=0.0,
                                    op0=OP.subtract, op1=OP.max)
            base = blk.tile([128, 256], F32, name="base")
            nc.vector.tensor_tensor(out=base, in0=x0, in1=dimm2, op=OP.min)
            tt = blk.tile([128, 256], F32, name="tt")
            nc.vector.tensor_tensor(out=tt, in0=x, in1=base, op=OP.subtract)
            # ns0 = -(weight of px at base)   = -max(min(1-t, 1+t), 0)
            # ns1 = -(weight of px at base+1) = -max(min(2-t, t), 0)
            # (1-|t| = min(1-t, 1+t); 1-|t-1| = min(2-t, t))
            a_t = blk.tile([128, 256], F32, name="a_t")
            nc.vector.tensor_scalar(out=a_t, in0=tt, scalar1=-1.0, scalar2=1.0,
                                    op0=OP.mult, op1=OP.add)
            b_t = blk.tile([128, 256], F32, name="b_t")
            nc.vector.tensor_scalar(out=b_t, in0=tt, scalar1=1.0, scalar2=None,
                                    op0=OP.add)
            ns0 = blk.tile([128, 256], F32, name="ns0")
            nc.vector.tensor_tensor(out=ns0, in0=a_t, in1=b_t, op=OP.min)
            nc.vector.tensor_scalar(out=ns0, in0=ns0, scalar1=-1.0, scalar2=0.0,
                                    op0=OP.mult, op1=OP.min)
            d_t = blk.tile([128, 256], F32, name="d_t")
            nc.vector.tensor_scalar(out=d_t, in0=tt, scalar1=-1.0, scalar2=2.0,
                                    op0=OP.mult, op1=OP.add)
            ns1 = blk.tile([128, 256], F32, name="ns1")
            nc.vector.tensor_tensor(out=ns1, in0=d_t, in1=tt, op=OP.min)
            nc.vector.tensor_scalar(out=ns1, in0=ns1, scalar1=-1.0, scalar2=0.0,
                                    op0=OP.mult, op1=OP.min)

            # ---- gather row index per (m,l,p): lsi_l + basey*W_l + basex ----
            b_ev = base[:].rearrange("p (k x) -> p k x", x=2)[:, :, 0]
            b_od = base[:].rearrange("p (k x) -> p k x", x=2)[:, :, 1]
            y0off = blk.tile([128, 128], F32, name="y0off")
            nc.vector.tensor_tensor(out=y0off, in0=b_od, in1=w8, op=OP.mult)
            nc.vector.tensor_tensor(out=y0off, in0=y0off, in1=c0, op=OP.add)
            nc.vector.tensor_tensor(out=y0off, in0=y0off, in1=b_ev, op=OP.add)
            offs16 = blk.tile([128, 128], I16, name="offs16")
            nc.vector.tensor_copy(out=offs16, in_=y0off)

            # fold partitions (q = qhi*16+qlo) -> [16 qlo, qhi 8, (m,pt) 128]
            # (issued on the ACT HWDGE queue to keep SP free for loads/stores)
            fold1 = blk.tile([16, 8, 128], I16, name="fold1")
            for qhi in range(8):
                nc.scalar.dma_start(
                    out=fold1[:, qhi, :],
                    in_=offs16[qhi * 16:(qhi + 1) * 16, :])
            # free-dim transpose (qhi, m, pt) -> (m, pt, qhi), rows 0..15
            fold2r = blk.tile([128, M * NPT * 8], I16, name="fold2r")
            nc.vector.tensor_copy(
                out=fold2r[0:16, :].rearrange("p (m t q) -> p m t q", m=M, t=NPT, q=8),
                in_=fold1[:].rearrange("p q (m t) -> p m t q", m=M, t=NPT))
            # replicate rows 0..15 to all 128 partitions
            nc.scalar.dma_start(out=fold2r[16:32, :], in_=fold2r[0:16, :])
            nc.scalar.dma_start(out=fold2r[32:64, :], in_=fold2r[0:32, :])
            nc.scalar.dma_start(out=fold2r[64:128, :], in_=fold2r[0:64, :])

            # ---- combined sample weights, pair-duplicated bf16:
            # swp[q, (m,pt), px, 0:2] = attw * s_y * s_x  (same value twice)
            tmp0 = blk.tile([128, 128], F32, name="tmp0")
            tmp1 = blk.tile([128, 128], F32, name="tmp1")
            nc.vector.tensor_tensor(out=tmp0, in0=attw, in1=ns0[:].rearrange(
                "p (k x) -> p k x", x=2)[:, :, 1], op=OP.mult)
            nc.vector.tensor_tensor(out=tmp1, in0=attw, in1=ns1[:].rearrange(
                "p (k x) -> p k x", x=2)[:, :, 1], op=OP.mult)
            ns0_ev = ns0[:].rearrange("p (k x) -> p k x", x=2)[:, :, 0]
            ns1_ev = ns1[:].rearrange("p (k x) -> p k x", x=2)[:, :, 0]
            swp = blk.tile([128, 128, 4, 2], BF16, name="swp")
            for px, (tmp, nsx) in enumerate(((tmp0, ns0_ev), (tmp0, ns1_ev),
                                             (tmp1, ns0_ev), (tmp1, ns1_ev))):
                nc.vector.tensor_tensor(
                    out=swp[:, :, px, :],
                    in0=tmp[:, :, None].broadcast_to([128, 128, 2]),
                    in1=nsx[:, :, None].broadcast_to([128, 128, 2]),
                    op=OP.mult)
            return qf_t, fold2r, swp

        def back(ib, state):
            q0 = ib * 128
            qf_t, fold2r, swp = state
            # ---- gather + blend per head ----
            sampled = blk.tile([128, DM], BF16, name="sampled")
            for m in range(M):
                g2m = big.tile([128, NPT, 4 * DH], BF16, name="g2m")
                # gather ucode caps at 1024 indices per call -> 2 calls/head
                for hf in range(2):
                    nc.gpsimd.dma_gather(
                        out_ap=g2m[:, hf * 8:(hf + 1) * 8, :],
                        in_ap=value4[:, m, :],
                        idxs_ap=fold2r[:, m * NPT * 8 + hf * 64:
                                       m * NPT * 8 + (hf + 1) * 64],
                        num_idxs=1024, num_idxs_reg=1024,
                        elem_size=4 * DH, elem_step=M * 4 * DH,
                        transpose=False, queue_num=0)
                # weighted multiply in 2x mode: every operand's innermost AP
                # dim is a stride-1 pair (ch2); ch-broadcast sits mid-AP.
                # wtm layout [cp(16), t(16), px(4), c2(2)], ch = cp*2+c2
                wtm = big.tile([128, 16, NPT, 4, 2], BF16, name="wtm")
                nc.vector.tensor_tensor(
                    out=wtm[:].rearrange("p cp t x c2 -> p t x cp c2"),
                    in0=g2m[:].rearrange("p t (x cp c2) -> p t x cp c2",
                                         x=4, cp=16, c2=2),
                    in1=swp[:, m * NPT:(m + 1) * NPT, :, None, :]
                        .broadcast_to([128, NPT, 4, 16, 2]),
                    op=OP.mult)
                # bf16 add-tree over points, then f32 reduce over (t2, px)
                wt1 = big.tile([128, 16, 8, 4, 2], BF16, name="wt1")
                nc.vector.tensor_tensor(out=wt1, in0=wtm[:, :, 0:8], in1=wtm[:, :, 8:16],
                                        op=OP.add)
                wt2 = big.tile([128, 16, 4, 4, 2], BF16, name="wt2")
                nc.vector.tensor_tensor(out=wt2, in0=wt1[:, :, 0:4], in1=wt1[:, :, 4:8],
                                        op=OP.add)
                wt3 = big.tile([128, 16, 2, 4, 2], BF16, name="wt3")
                nc.vector.tensor_tensor(out=wt3, in0=wt2[:, :, 0:2], in1=wt2[:, :, 2:4],
                                        op=OP.add)
                with nc.allow_low_precision(reason="bf16 blend within 2e-2 tol"):
                    nc.vector.tensor_reduce(
                        out=sampled[:, m * DH:(m + 1) * DH]
                            .rearrange("p (cp c2) -> p cp c2", c2=2),
                        in_=wt3[:].rearrange("p cp t x c2 -> p cp c2 (t x)"),
                        axis=AX.X, op=OP.add)

            # ---- output projection ----
            # transpose via plain matmul against identity (bf16 lhsT, f32
            # PSUM out) — nc.tensor.transpose would demand a bf16 PSUM tile
            sT = []
            for c in range(2):
                tp = ps_t.tile([128, 128], F32, name="s_tp", tag="tp")
                nc.tensor.matmul(out=tp, lhsT=sampled[:, c * 128:(c + 1) * 128],
                                 rhs=ident, start=True, stop=True)
                sb = blk.tile([128, 128], BF16, name="sT")
                nc.vector.tensor_copy(out=sb, in_=tp)
                sT.append(sb)
            ps_h = ps_mm.tile([128, DM], F32, name="ps_h", tag="mm")
            for c in range(2):
                nc.tensor.matmul(out=ps_h, lhsT=sT[c], rhs=Wout[c], start=(c == 0), stop=False)
            nc.tensor.matmul(out=ps_h, lhsT=ones1, rhs=bout, start=False, stop=True)

            # ---- LN1 ----
            r1 = blk.tile([128, DM], F32, name="r1")
            nc.vector.tensor_tensor(out=r1, in0=qf_t, in1=ps_h, op=OP.add)
            h = _layernorm(nc, blk, r1, ln1g, ln1b, eps_t)

            # ---- FFN ----
            hT = []
            for c in range(2):
                tp = ps_t.tile([128, 128], F32, name="h_tp", tag="tp")
                nc.tensor.transpose(out=tp, in_=h[:, c * 128:(c + 1) * 128], identity=identf)
                sb = blk.tile([128, 128], BF16, name="hT")
                nc.vector.tensor_copy(out=sb, in_=tp)
                hT.append(sb)
            relu1 = []
            for fc in range(8):
                pf = ps_f1.tile([128, 128], F32, name="pf")
                for c in range(2):
                    nc.tensor.matmul(out=pf, lhsT=W1[c][:, fc * 128:(fc + 1) * 128],
                                     rhs=hT[c], start=(c == 0), stop=False)
                nc.tensor.matmul(out=pf, lhsT=b1[:, fc * 128:(fc + 1) * 128],
                                 rhs=ones1, start=False, stop=True)
                rt = blk.tile([128, 128], BF16, name=f"relu1_{fc}")
                nc.scalar.activation(out=rt, in_=pf, func=AF.Relu)
                relu1.append(rt)
            ps_o = ps_mm.tile([128, DM], F32, name="ps_o", tag="mm")
            for fc in range(8):
                nc.tensor.matmul(out=ps_o, lhsT=relu1[fc], rhs=W2[fc], start=(fc == 0), stop=False)
            nc.tensor.matmul(out=ps_o, lhsT=ones1, rhs=b2, start=False, stop=True)

            # ---- LN2 + store ----
            r2 = blk.tile([128, DM], F32, name="r2")
            nc.vector.tensor_tensor(out=r2, in0=h, in1=ps_o, op=OP.add)
            o = _layernorm(nc, blk, r2, ln2g, ln2b, eps_t)
            nc.sync.dma_start(out=out_ap[q0:q0 + 128, :], in_=o)

        state = {}
        for ib in range(NBLK + 1):
            if ib < NBLK:
                state[ib] = front(ib)
            if ib >= 1:
                back(ib - 1, state.pop(ib - 1))


def _layernorm(nc, pool, r, g, b, eps_t):
    stats = pool.tile([128, 6], F32, name="ln_stats")
    nc.vector.bn_stats(out=stats, in_=r)
    mv = pool.tile([128, 2], F32, name="ln_mv")
    nc.vector.bn_aggr(out=mv, in_=stats)
    rstd = pool.tile([128, 1], F32, name="ln_rstd")
    nc.scalar.activation(out=rstd, in_=mv[:, 1:2], func=AF.Sqrt, bias=eps_t)
    nc.vector.reciprocal(out=rstd, in_=rstd)
    xs = pool.tile([128, DM], F32, name="ln_xs")
    nc.vector.tensor_scalar(out=xs, in0=r, scalar1=mv[:, 0:1], scalar2=rstd,
                            op0=OP.subtract, op1=OP.mult)
    h = pool.tile([128, DM], F32, name="ln_h")
    nc.vector.tensor_tensor(out=h, in0=xs, in1=g, op=OP.mult)
    nc.vector.tensor_tensor(out=h, in0=h, in1=b, op=OP.add)
    return h


# ---------------------------------------------------------------------------
# host side
# ---------------------------------------------------------------------------

_prog_cache = {}


def _get_program():
    if "nc" not in _prog_cache:
        _prog_cache["nc"] = build_program()
    return _prog_cache["nc"]


def _host_constants(ss, lsi):
    ss = np.asarray(ss, np.int64)
    lsi = np.asarray(lsi, np.int64)
    f = np.float32
    H = ss[:, 0].astype(np.int64)
    W = ss[:, 1].astype(np.int64)
    refd = np.zeros((LV, 2), f)
    refd[:, 0] = W; refd[:, 1] = H
    refdims = np.broadcast_to(refd.reshape(1, -1), (128, LV * 2)).copy()
    dm2 = np.zeros((M, LV, P, 2), f)
    dm2[:, :, :, 0] = (W - 2)[None, :, None]
    dm2[:, :, :, 1] = (H - 2)[None, :, None]
    dimm2 = np.broadcast_to(dm2.reshape(1, -1), (128, M * LV * P * 2)).copy()
    w8a = np.zeros((M, LV, P), f)
    w8a[:, :, :] = W[None, :, None]
    w8 = np.broadcast_to(w8a.reshape(1, -1), (128, M * LV * P)).copy()
    c0a = np.zeros((M, LV, P), f)
    c0a[:, :, :] = lsi[None, :, None]
    c0 = np.broadcast_to(c0a.reshape(1, -1), (128, M * LV * P)).copy()
    return refdims, dimm2, w8, c0


def _build_in_maps(inputs):
    src = np.asarray(inputs["src"], np.float32)
    q_feat = np.asarray(inputs["q_feat"], np.float32)
    q_pos = np.asarray(inputs["q_pos"], np.float32)
    ref = np.asarray(inputs["reference_points"], np.float32)
    ss = np.asarray(inputs["spatial_shapes"])
    lsi_in = np.asarray(inputs["level_start_index"])

    lsi = lsi_in.astype(np.int64)

    assert src.shape == (B, L, DM), src.shape
    refdims, dimm2, w8, c0 = _host_constants(ss, lsi)

    def as_bf16(a):
        import ml_dtypes
        return np.asarray(a, np.float32).astype(ml_dtypes.bfloat16)

    import ml_dtypes
    common = {
        "Woff": as_bf16(inputs["W_off"]),
        "Wattn": as_bf16(inputs["W_attn"]),
        "Wv": as_bf16(inputs["W_v"]),
        "Wout": as_bf16(inputs["W_out"]),
        "W1": as_bf16(inputs["W1"]),
        "W2": as_bf16(inputs["W2"]),
        "boff": as_bf16(inputs["b_off"]).reshape(1, -1),
        "battn": as_bf16(inputs["b_attn"]).reshape(1, -1),
        "bv": as_bf16(inputs["b_v"]).reshape(1, -1),
        "bout": as_bf16(inputs["b_out"]).reshape(1, -1),
        "b1": as_bf16(inputs["b1"]).reshape(1, -1),
        "b2": as_bf16(inputs["b2"]).reshape(1, -1),
        "ln1g": np.broadcast_to(np.asarray(inputs["ln1_g"], np.float32), (128, DM)).copy(),
        "ln1b": np.broadcast_to(np.asarray(inputs["ln1_b"], np.float32), (128, DM)).copy(),
        "ln2g": np.broadcast_to(np.asarray(inputs["ln2_g"], np.float32), (128, DM)).copy(),
        "ln2b": np.broadcast_to(np.asarray(inputs["ln2_b"], np.float32), (128, DM)).copy(),
        "ident": np.eye(128, dtype=np.float32).astype(ml_dtypes.bfloat16),
        "identf": np.eye(128, dtype=np.float32),
        "ones1": as_bf16(np.ones((1, 128), np.float32)),
        "refdims": refdims, "dimm2": dimm2, "w8": w8, "c0": c0,
    }

    halves = [(0, LC), (LC, L - LC)]
    in_maps = []
    for core in range(8):
        b = core // 2
        h0, hn = halves[core % 2]
        qf = np.zeros((LCPAD, DM), np.float32)
        qp = np.zeros((LCPAD, DM), np.float32)
        rf = np.zeros((LCPAD, LV, 2), np.float32)
        qf[:hn] = q_feat[b, h0:h0 + hn]
        qp[:hn] = q_pos[b, h0:h0 + hn]
        rf[:hn] = ref[b, h0:h0 + hn]
        m = dict(common)
        m.update({"qf": qf, "qp": qp, "ref": rf, "srcb": src[b]})
        in_maps.append(m)
    return in_maps


def kernel(**inputs):
    from concourse.bass_utils import run_bass_kernel_spmd

    in_maps = _build_in_maps(inputs)
    nc = _get_program()
    res = run_bass_kernel_spmd(nc, in_maps, core_ids=list(range(8)))

    halves = [(0, LC), (LC, L - LC)]
    out = np.zeros((B, L, DM), np.float32)
    for core in range(8):
        b = core // 2
        h0, hn = halves[core % 2]
        out[b, h0:h0 + hn] = res.results[core]["out"][:hn]
    return out


# revision 28
# speedup vs baseline: 2.0375x; 1.0661x over previous
"""Deformable Transformer Encoder Layer — Trainium2 Bass kernel (v2).

Sharding: 8 cores = (batch b in 0..3) x (query-half h in 0..1).
Each core computes the full layer for its (b, query-half) slice.

v2 changes vs v1:
  - value stored bf16, quad-duplicated: value4[r, m, 0:128] =
    [v[r], v[r+1], v[r+W_l], v[r+W_l+1]] (W_l = level width of row r's
    level), so one 256B gather fetch covers a full 2x2 bilinear patch.
    One dma_gather per (block, head): 2048 idxs (vs 4x1024 per head
    with 2 fetches/point in v1) -> half the DMA descriptors, 1/4 the
    SWDGE calls.
  - bilinear edge weights via s0 = -min(|x-base|-1, 0),
    s1 = -min(|x-base-1|-1, 0) (negations cancel in products):
    8 DVE ops/block instead of 17.
  - blend in bf16 with pair-duplicated weights (innermost stride-1
    pair) so the big multiply runs in DVE 2x mode; reduction is a bf16
    add-tree + small f32 tensor_reduce.
"""

import os
import sys
import numpy as np

for _p in ("/opt/trn_rl_repo", "/root/.axon_site/_ro/trn_rl_repo"):
    if os.path.isdir(_p) and _p not in sys.path:
        sys.path.insert(0, _p)

import concourse.bass as bass
import concourse.mybir as mybir
import concourse.tile as tile
from concourse import bacc
from concourse.bass import AP

F32 = mybir.dt.float32
BF16 = mybir.dt.bfloat16
I32 = mybir.dt.int32
I16 = mybir.dt.int16
AF = mybir.ActivationFunctionType
OP = mybir.AluOpType
AX = mybir.AxisListType

# Problem constants (checked against inputs at runtime on host).
M, LV, P, DM, DH, DF = 8, 4, 4, 256, 32, 1024
NPT = LV * P        # 16 sample points per (query, head)
L = 11253
B = 4
LC = 5627           # queries per core (split [5627, 5626])
LCPAD = 5632        # 44 * 128
NBLK = LCPAD // 128
RPAD = 11264        # value rows padded (88 * 128)
EPS = 1e-5
TWO23 = 12582912.0   # 3*2^22: x + (TWO23-0.5) lands in [2^23, 2^24) where ulp=1
SHAPES = [(92, 92), (46, 46), (23, 23), (12, 12)]
LSI = [0, 8464, 10580, 11109]
NQUEUES = 1


def build_program():
    nc = bacc.Bacc("TRN2", target_bir_lowering=False, debug=False, enable_asserts=False,
                   num_swdge_queues=NQUEUES)

    t = {}
    def inp(name, shape, dtype=F32):
        t[name] = nc.dram_tensor(name, list(shape), dtype, kind="ExternalInput").ap()
        return t[name]

    # per-core data
    inp("qf", (LCPAD, DM)); inp("qp", (LCPAD, DM)); inp("ref", (LCPAD, LV, 2))
    inp("srcb", (L, DM))
    # weights (bf16 on host for matmul rhs)
    inp("Woff", (DM, M * NPT * 2), BF16)
    inp("Wattn", (DM, M * NPT), BF16)
    inp("Wv", (DM, DM), BF16)
    inp("Wout", (DM, DM), BF16)
    inp("W1", (DM, DF), BF16)
    inp("W2", (DF, DM), BF16)
    # biases as [1, N] rows (rank-1 matmul trick), bf16
    inp("boff", (1, M * NPT * 2), BF16)
    inp("battn", (1, M * NPT), BF16)
    inp("bv", (1, DM), BF16)
    inp("bout", (1, DM), BF16)
    inp("b1", (1, DF), BF16)
    inp("b2", (1, DM), BF16)
    # layernorm params replicated across partitions (f32)
    inp("ln1g", (128, DM)); inp("ln1b", (128, DM))
    inp("ln2g", (128, DM)); inp("ln2b", (128, DM))
    # constants
    inp("ident", (128, 128), BF16)            # bf16 identity for PE transpose
    inp("identf", (128, 128))                 # f32 identity for PE transpose
    inp("ones1", (1, 128), BF16)              # rank-1 lhsT of ones
    inp("refdims", (128, LV * 2))             # (l,xy) -> W_l | H_l, replicated
    inp("dimm2", (128, M * NPT * 2))          # (m,l,p,xy) -> dim-2, replicated
    inp("w8", (128, M * NPT))                 # (m,l,p) -> W_l, replicated
    inp("c0", (128, M * NPT))                 # (m,l,p) -> lsi_l, replicated

    out_ap = nc.dram_tensor("out", [LCPAD, DM], F32, kind="ExternalOutput").ap()

    with tile.TileContext(nc) as tc:
        _build(tc, out_ap, t)

    nc.compile()
    return nc


def _build(tc, out_ap, t):
    nc = tc.nc
    from contextlib import ExitStack
    ctx = ExitStack()
    with ctx:
        consts = ctx.enter_context(tc.tile_pool(name="consts", bufs=1))
        wpool = ctx.enter_context(tc.tile_pool(name="wpool", bufs=1))
        vblk = ctx.enter_context(tc.tile_pool(name="vblk", bufs=3))
        blk = ctx.enter_context(tc.tile_pool(name="blk", bufs=3))
        big = ctx.enter_context(tc.tile_pool(name="big", bufs=4))
        ps_t = ctx.enter_context(tc.tile_pool(name="ps_t", bufs=2, space="PSUM"))
        ps_mm = ctx.enter_context(tc.tile_pool(name="ps_mm", bufs=3, space="PSUM"))
        ps_f1 = ctx.enter_context(tc.tile_pool(name="ps_f1", bufs=3, space="PSUM"))
        dram = ctx.enter_context(tc.tile_pool(name="dram", bufs=1, space="DRAM"))

        # ---- resident constants / weights in SBUF ----
        def ld(name):
            ap = t[name]
            tile_ = consts.tile(list(ap.shape), ap.dtype, name=name + "_s")
            nc.sync.dma_start(out=tile_, in_=ap)
            return tile_

        ident = ld("ident")
        identf = ld("identf")
        ones1 = ld("ones1")
        eps_t = consts.tile([128, 1], F32, name="eps_t")
        nc.vector.memset(eps_t, EPS)
        refdims = ld("refdims"); dimm2 = ld("dimm2"); w8 = ld("w8"); c0 = ld("c0")
        ln1g = ld("ln1g"); ln1b = ld("ln1b"); ln2g = ld("ln2g"); ln2b = ld("ln2b")
        boff = ld("boff"); battn = ld("battn"); bv = ld("bv")
        bout = ld("bout"); b1 = ld("b1"); b2 = ld("b2")

        def ldw(name, kchunks):
            ap = t[name]
            K, N = ap.shape
            tiles = []
            for k in range(kchunks):
                w_ = wpool.tile([128, N], ap.dtype, name=f"{name}_{k}")
                nc.sync.dma_start(out=w_, in_=ap[k * 128:(k + 1) * 128, :])
                tiles.append(w_)
            return tiles

        Woff = ldw("Woff", 2); Wattn = ldw("Wattn", 2); Wv = ldw("Wv", 2)
        Wout = ldw("Wout", 2); W1 = ldw("W1", 2); W2 = ldw("W2", 8)

        # ---- value projection -> DRAM scratch, quad-duplicated bf16:
        # value4[r, m, 0:32]=v[r,m]  [32:64]=v[r+1,m]
        #            [64:96]=v[r+W_l,m]  [96:128]=v[r+W_l+1,m]
        # (W_l = width of the level containing dst row r)
        value4 = dram.tile([RPAD, M, 4 * DH], BF16, name="value4")

        for vb in range((L + 127) // 128):
            p0 = vb * 128
            pn = min(128, L - p0)
            s_t = vblk.tile([128, DM], F32, name="s_t")
            nc.sync.dma_start(out=s_t[:pn], in_=t["srcb"][p0:p0 + pn, :])
            sT = []
            for c in range(2):
                tp = ps_t.tile([128, 128], F32, name="v_tp", tag="tp")
                nc.tensor.transpose(out=tp[:, :pn], in_=s_t[:pn, c * 128:(c + 1) * 128],
                                    identity=identf[:pn, :pn])
                sb = vblk.tile([128, 128], BF16, name="v_sT")
                nc.vector.tensor_copy(out=sb[:, :pn], in_=tp[:, :pn])
                sT.append(sb)
            pv = ps_mm.tile([128, DM], F32, name="v_ps", tag="mm")
            for c in range(2):
                nc.tensor.matmul(out=pv[:pn], lhsT=sT[c][:, :pn], rhs=Wv[c], start=(c == 0), stop=False)
            nc.tensor.matmul(out=pv[:pn], lhsT=ones1[:, :pn], rhs=bv, start=False, stop=True)
            vt = vblk.tile([128, DM], BF16, name="v_out")
            nc.vector.tensor_copy(out=vt[:pn], in_=pv[:pn])
            vt_v = vt[:].rearrange("p (m c) -> p m c", c=DH)
            # px0: value4[r,:,0:32] = v[r]
            nc.sync.dma_start(out=value4[p0:p0 + pn, :, 0:DH], in_=vt_v[:pn])
            # px1: value4[r-1,:,32:64] = v[r]
            if vb == 0:
                nc.scalar.dma_start(out=value4[0:pn - 1, :, DH:2 * DH], in_=vt_v[1:pn])
            else:
                nc.scalar.dma_start(out=value4[p0 - 1:p0 + pn - 1, :, DH:2 * DH], in_=vt_v[:pn])
            # px2/px3: value4[r-W,:,64:96] = v[r]; value4[r-W-1,:,96:128] = v[r]
            # (level-dependent W; a 128-row src block can straddle one level
            #  boundary -> up to 2 pieces per shift)
            for li, (H, W) in enumerate(SHAPES):
                lvl0 = LSI[li]
                lvl1 = LSI[li] + H * W
                for shift, o0 in ((W, 2 * DH), (W + 1, 3 * DH)):
                    # dst rows r (in level li) receiving src rows r+shift
                    # from this block: r in [lvl0, lvl1) & [p0-shift, p0+pn-shift)
                    d0 = max(lvl0, p0 - shift)
                    d1 = min(lvl1, p0 + pn - shift)
                    if d0 >= d1:
                        continue
                    s0_, s1_ = d0 + shift - p0, d1 + shift - p0
                    nc.scalar.dma_start(out=value4[d0:d1, :, o0:o0 + DH],
                                        in_=vt_v[s0_:s1_])

        # ---- main per-block loop, software-pipelined emission:
        # front(n+1) is emitted before back(n) so each engine's in-order
        # queue interleaves the independent stages of adjacent blocks
        # (avoids head-of-line blocking on PE/SP sequencers).
        def front(ib):
            q0 = ib * 128
            qf_t = blk.tile([128, DM], F32, name="qf_t")
            query = blk.tile([128, DM], F32, name="query")
            ref_t = blk.tile([128, LV, 2], F32, name="ref_t")
            nc.sync.dma_start(out=qf_t, in_=t["qf"][q0:q0 + 128, :])
            nc.sync.dma_start(out=query, in_=t["qp"][q0:q0 + 128, :])
            nc.sync.dma_start(out=ref_t, in_=t["ref"][q0:q0 + 128, :, :])

            # transpose query -> qT bf16 chunks
            qT = []
            for c in range(2):
                tp = ps_t.tile([128, 128], F32, name="q_tp", tag="tp")
                nc.tensor.transpose(out=tp, in_=query[:, c * 128:(c + 1) * 128], identity=identf)
                sb = blk.tile([128, 128], BF16, name="qT")
                nc.vector.tensor_copy(out=sb, in_=tp)
                qT.append(sb)

            # offsets projection [128q, 256]
            ps_off = ps_mm.tile([128, 256], F32, name="ps_off", tag="mm")
            for c in range(2):
                nc.tensor.matmul(out=ps_off, lhsT=qT[c], rhs=Woff[c], start=(c == 0), stop=False)
            nc.tensor.matmul(out=ps_off, lhsT=ones1, rhs=boff, start=False, stop=True)

            # attention weights projection + softmax over (l,p) per head
            ps_at = ps_mm.tile([128, 128], F32, name="ps_at", tag="mm")
            for c in range(2):
                nc.tensor.matmul(out=ps_at, lhsT=qT[c], rhs=Wattn[c], start=(c == 0), stop=False)
            nc.tensor.matmul(out=ps_at, lhsT=ones1, rhs=battn, start=False, stop=True)
            expt = blk.tile([128, 128], F32, name="expt")
            nc.scalar.activation(out=expt, in_=ps_at, func=AF.Exp)
            den = blk.tile([128, M], F32, name="den")
            nc.vector.tensor_reduce(out=den, in_=expt[:].rearrange("p (m k) -> p m k", m=M),
                                    axis=AX.X, op=OP.add)
            nc.vector.reciprocal(out=den, in_=den)
            attw = blk.tile([128, 128], F32, name="attw")
            nc.vector.tensor_tensor(out=attw[:].rearrange("p (m k) -> p m k", m=M),
                                    in0=expt[:].rearrange("p (m k) -> p m k", m=M),
                                    in1=den[:, :, None].broadcast_to([128, M, NPT]),
                                    op=OP.mult)

            # ---- sampling math (all [128, 256] tiles over (m,l,p,xy)) ----
            # x = ref*dims - 0.5 + off   (offset normalizer cancels: off/W_norm*W = off)
            refx = blk.tile([128, LV * 2], F32, name="refx")
            nc.vector.tensor_tensor(out=refx, in0=ref_t[:].rearrange("p l x -> p (l x)"),
                                    in1=refdims, op=OP.mult)
            nc.vector.tensor_scalar(out=refx, in0=refx, scalar1=0.5, scalar2=None, op0=OP.subtract)
            refx32 = blk.tile([128, LV * P * 2], F32, name="refx32")
            nc.vector.tensor_copy(
                out=refx32[:].rearrange("p (l q x) -> p l q x", l=LV, q=P),
                in_=refx[:].rearrange("p (l x) -> p l x", x=2)[:, :, None, :]
                    .broadcast_to([128, LV, P, 2]))
            x = blk.tile([128, 256], F32, name="x")
            nc.vector.tensor_tensor(
                out=x[:].rearrange("p (m k) -> p m k", m=M),
                in0=ps_off[:].rearrange("p (m k) -> p m k", m=M),
                in1=refx32[:, None, :].broadcast_to([128, M, NPT * 2]),
                op=OP.add)
            # base = clamp(floor(x), 0, dim-2); t = x - base
            # floor via round(x-0.5): ((x-0.5)+C)-C with C=3*2^22 (fuse max-0)
            x0 = blk.tile([128, 256], F32, name="x0")
            nc.vector.tensor_scalar(out=x0, in0=x, scalar1=0.5, scalar2=TWO23,
                                    op0=OP.subtract, op1=OP.add)
            nc.vector.tensor_scalar(out=x0, in0=x0, scalar1=TWO23, scalar2=0.0,
                                    op0=OP.subtract, op1=OP.max)
            base = blk.tile([128, 256], F32, name="base")
            nc.vector.tensor_tensor(out=base, in0=x0, in1=dimm2, op=OP.min)
            # bf16 edge-weight chain (2x DVE mode; index path stays f32)
            tt = blk.tile([128, 256], BF16, name="tt")
            nc.vector.tensor_tensor(out=tt, in0=x, in1=base, op=OP.subtract)
            # ns0 = -(weight of px at base)   = -max(min(1-t, 1+t), 0)
            # ns1 = -(weight of px at base+1) = -max(min(2-t, t), 0)
            # (1-|t| = min(1-t, 1+t); 1-|t-1| = min(2-t, t))
            a_t = blk.tile([128, 256], BF16, name="a_t")
            nc.vector.tensor_scalar(out=a_t, in0=tt, scalar1=-1.0, scalar2=1.0,
                                    op0=OP.mult, op1=OP.add)
            b_t = blk.tile([128, 256], BF16, name="b_t")
            nc.vector.tensor_scalar(out=b_t, in0=tt, scalar1=1.0, scalar2=None,
                                    op0=OP.add)
            ns0 = blk.tile([128, 256], BF16, name="ns0")
            nc.vector.tensor_tensor(out=ns0, in0=a_t, in1=b_t, op=OP.min)
            nc.vector.tensor_scalar(out=ns0, in0=ns0, scalar1=-1.0, scalar2=0.0,
                                    op0=OP.mult, op1=OP.min)
            d_t = blk.tile([128, 256], BF16, name="d_t")
            nc.vector.tensor_scalar(out=d_t, in0=tt, scalar1=-1.0, scalar2=2.0,
                                    op0=OP.mult, op1=OP.add)
            ns1 = blk.tile([128, 256], BF16, name="ns1")
            nc.vector.tensor_tensor(out=ns1, in0=d_t, in1=tt, op=OP.min)
            nc.vector.tensor_scalar(out=ns1, in0=ns1, scalar1=-1.0, scalar2=0.0,
                                    op0=OP.mult, op1=OP.min)

            # ---- gather row index per (m,l,p): lsi_l + basey*W_l + basex ----
            b_ev = base[:].rearrange("p (k x) -> p k x", x=2)[:, :, 0]
            b_od = base[:].rearrange("p (k x) -> p k x", x=2)[:, :, 1]
            y0off = blk.tile([128, 128], F32, name="y0off")
            nc.vector.tensor_tensor(out=y0off, in0=b_od, in1=w8, op=OP.mult)
            nc.vector.tensor_tensor(out=y0off, in0=y0off, in1=c0, op=OP.add)
            nc.vector.tensor_tensor(out=y0off, in0=y0off, in1=b_ev, op=OP.add)
            offs16 = blk.tile([128, 128], I16, name="offs16")
            nc.vector.tensor_copy(out=offs16, in_=y0off)

            # fold partitions (q = qhi*16+qlo) -> [16 qlo, qhi 8, (m,pt) 128]
            # (issued on the ACT HWDGE queue to keep SP free for loads/stores)
            fold1 = blk.tile([16, 8, 128], I16, name="fold1")
            for qhi in range(8):
                nc.scalar.dma_start(
                    out=fold1[:, qhi, :],
                    in_=offs16[qhi * 16:(qhi + 1) * 16, :])
            # free-dim transpose (qhi, m, pt) -> (m, pt, qhi), rows 0..15
            # (on GPSIMD: DVE is the bottleneck engine, Pool has slack)
            fold2r = blk.tile([128, M * NPT * 8], I16, name="fold2r")
            nc.gpsimd.tensor_copy(
                out=fold2r[0:16, :].rearrange("p (m t q) -> p m t q", m=M, t=NPT, q=8),
                in_=fold1[:].rearrange("p q (m t) -> p m t q", m=M, t=NPT))
            # replicate rows 0..15 to all 128 partitions
            nc.scalar.dma_start(out=fold2r[16:32, :], in_=fold2r[0:16, :])
            nc.scalar.dma_start(out=fold2r[32:64, :], in_=fold2r[0:32, :])
            nc.scalar.dma_start(out=fold2r[64:128, :], in_=fold2r[0:64, :])

            # ---- combined sample weights, pair-duplicated bf16:
            # swp[q, (m,pt), px, 0:2] = attw * s_y * s_x  (same value twice)
            tmp0 = blk.tile([128, 128], F32, name="tmp0")
            tmp1 = blk.tile([128, 128], F32, name="tmp1")
            nc.vector.tensor_tensor(out=tmp0, in0=attw, in1=ns0[:].rearrange(
                "p (k x) -> p k x", x=2)[:, :, 1], op=OP.mult)
            nc.vector.tensor_tensor(out=tmp1, in0=attw, in1=ns1[:].rearrange(
                "p (k x) -> p k x", x=2)[:, :, 1], op=OP.mult)
            ns0_ev = ns0[:].rearrange("p (k x) -> p k x", x=2)[:, :, 0]
            ns1_ev = ns1[:].rearrange("p (k x) -> p k x", x=2)[:, :, 0]
            swp = blk.tile([128, 128, 4, 2], BF16, name="swp")
            for px, (tmp, nsx) in enumerate(((tmp0, ns0_ev), (tmp0, ns1_ev),
                                             (tmp1, ns0_ev), (tmp1, ns1_ev))):
                nc.vector.tensor_tensor(
                    out=swp[:, :, px, :],
                    in0=tmp[:, :, None].broadcast_to([128, 128, 2]),
                    in1=nsx[:, :, None].broadcast_to([128, 128, 2]),
                    op=OP.mult)
            return qf_t, fold2r, swp

        def back(ib, state):
            q0 = ib * 128
            qf_t, fold2r, swp = state
            # ---- gather + blend per head ----
            sampled = blk.tile([128, DM], BF16, name="sampled")
            for m in range(M):
                g2m = big.tile([128, NPT, 4 * DH], BF16, name="g2m")
                # gather ucode caps at 1024 indices per call -> 2 calls/head
                for hf in range(2):
                    nc.gpsimd.dma_gather(
                        out_ap=g2m[:, hf * 8:(hf + 1) * 8, :],
                        in_ap=value4[:, m, :],
                        idxs_ap=fold2r[:, m * NPT * 8 + hf * 64:
                                       m * NPT * 8 + (hf + 1) * 64],
                        num_idxs=1024, num_idxs_reg=1024,
                        elem_size=4 * DH, elem_step=M * 4 * DH,
                        transpose=False, queue_num=0)
                # weighted multiply in 2x mode: every operand's innermost AP
                # dim is a stride-1 pair (ch2); ch-broadcast sits mid-AP.
                # wtm layout [cp(16), t(16), px(4), c2(2)], ch = cp*2+c2
                wtm = big.tile([128, 16, NPT, 4, 2], BF16, name="wtm")
                nc.vector.tensor_tensor(
                    out=wtm[:].rearrange("p cp t x c2 -> p t x cp c2"),
                    in0=g2m[:].rearrange("p t (x cp c2) -> p t x cp c2",
                                         x=4, cp=16, c2=2),
                    in1=swp[:, m * NPT:(m + 1) * NPT, :, None, :]
                        .broadcast_to([128, NPT, 4, 16, 2]),
                    op=OP.mult)
                # bf16 add-tree over points, then f32 reduce over (t2, px)
                wt1 = big.tile([128, 16, 8, 4, 2], BF16, name="wt1")
                nc.vector.tensor_tensor(out=wt1, in0=wtm[:, :, 0:8], in1=wtm[:, :, 8:16],
                                        op=OP.add)
                wt2 = big.tile([128, 16, 4, 4, 2], BF16, name="wt2")
                nc.vector.tensor_tensor(out=wt2, in0=wt1[:, :, 0:4], in1=wt1[:, :, 4:8],
                                        op=OP.add)
                wt3 = big.tile([128, 16, 2, 4, 2], BF16, name="wt3")
                nc.vector.tensor_tensor(out=wt3, in0=wt2[:, :, 0:2], in1=wt2[:, :, 2:4],
                                        op=OP.add)
                wt4 = big.tile([128, 16, 1, 4, 2], BF16, name="wt4")
                nc.vector.tensor_tensor(out=wt4, in0=wt3[:, :, 0:1], in1=wt3[:, :, 1:2],
                                        op=OP.add)
                with nc.allow_low_precision(reason="bf16 blend within 2e-2 tol"):
                    nc.vector.tensor_reduce(
                        out=sampled[:, m * DH:(m + 1) * DH]
                            .rearrange("p (cp c2) -> p cp c2", c2=2),
                        in_=wt4[:].rearrange("p cp t x c2 -> p cp c2 (t x)"),
                        axis=AX.X, op=OP.add)

            # ---- output projection ----
            # transpose via plain matmul against identity (bf16 lhsT, f32
            # PSUM out) — nc.tensor.transpose would demand a bf16 PSUM tile
            sT = []
            for c in range(2):
                tp = ps_t.tile([128, 128], F32, name="s_tp", tag="tp")
                nc.tensor.matmul(out=tp, lhsT=sampled[:, c * 128:(c + 1) * 128],
                                 rhs=ident, start=True, stop=True)
                sb = blk.tile([128, 128], BF16, name="sT")
                nc.vector.tensor_copy(out=sb, in_=tp)
                sT.append(sb)
            ps_h = ps_mm.tile([128, DM], F32, name="ps_h", tag="mm")
            for c in range(2):
                nc.tensor.matmul(out=ps_h, lhsT=sT[c], rhs=Wout[c], start=(c == 0), stop=False)
            nc.tensor.matmul(out=ps_h, lhsT=ones1, rhs=bout, start=False, stop=True)

            # ---- LN1 (affine folded: W1 = diag(ln1_g) @ W1, b1 += ln1_b @ W1,
            # b2 += ln1_b host-side; FFN consumes the normalized xs, the
            # residual applies xs * ln1_g; the +ln1_b lands via b2) ----
            r1 = blk.tile([128, DM], F32, name="r1")
            nc.vector.tensor_tensor(out=r1, in0=qf_t, in1=ps_h, op=OP.add)
            xs1 = _ln_normalize(nc, blk, r1, eps_t, "l1")

            # ---- FFN ----
            hT = []
            for c in range(2):
                tp = ps_t.tile([128, 128], F32, name="h_tp", tag="tp")
                nc.tensor.transpose(out=tp, in_=xs1[:, c * 128:(c + 1) * 128], identity=identf)
                sb = blk.tile([128, 128], BF16, name="hT")
                nc.vector.tensor_copy(out=sb, in_=tp)
                hT.append(sb)
            relu1 = []
            for fc in range(8):
                pf = ps_f1.tile([128, 128], F32, name="pf")
                for c in range(2):
                    nc.tensor.matmul(out=pf, lhsT=W1[c][:, fc * 128:(fc + 1) * 128],
                                     rhs=hT[c], start=(c == 0), stop=False)
                nc.tensor.matmul(out=pf, lhsT=b1[:, fc * 128:(fc + 1) * 128],
                                 rhs=ones1, start=False, stop=True)
                rt = blk.tile([128, 128], BF16, name=f"relu1_{fc}")
                nc.scalar.activation(out=rt, in_=pf, func=AF.Relu)
                relu1.append(rt)
            ps_o = ps_mm.tile([128, DM], F32, name="ps_o", tag="mm")
            for fc in range(8):
                nc.tensor.matmul(out=ps_o, lhsT=relu1[fc], rhs=W2[fc], start=(fc == 0), stop=False)
            nc.tensor.matmul(out=ps_o, lhsT=ones1, rhs=b2, start=False, stop=True)

            # ---- LN2 + store ----
            r2 = blk.tile([128, DM], F32, name="r2")
            nc.vector.tensor_tensor(out=r2, in0=xs1, in1=ln1g, op=OP.mult)
            nc.vector.tensor_tensor(out=r2, in0=r2, in1=ps_o, op=OP.add)
            xs2 = _ln_normalize(nc, blk, r2, eps_t, "l2")
            o = blk.tile([128, DM], F32, name="o_t")
            nc.vector.tensor_tensor(out=o, in0=xs2, in1=ln2g, op=OP.mult)
            nc.vector.tensor_tensor(out=o, in0=o, in1=ln2b, op=OP.add)
            nc.sync.dma_start(out=out_ap[q0:q0 + 128, :], in_=o)

        state = {}
        for ib in range(NBLK + 1):
            if ib < NBLK:
                state[ib] = front(ib)
            if ib >= 1:
                back(ib - 1, state.pop(ib - 1))


def _ln_normalize(nc, pool, r, eps_t, tag):
    """(r - mean) * rsqrt(var + eps); the normalize applied on ACT."""
    stats = pool.tile([128, 6], F32, name=f"ln_stats_{tag}")
    nc.vector.bn_stats(out=stats, in_=r)
    mv = pool.tile([128, 2], F32, name=f"ln_mv_{tag}")
    nc.vector.bn_aggr(out=mv, in_=stats)
    rstd = pool.tile([128, 1], F32, name=f"ln_rstd_{tag}")
    nc.scalar.activation(out=rstd, in_=mv[:, 1:2], func=AF.Sqrt, bias=eps_t)
    nc.vector.reciprocal(out=rstd, in_=rstd)
    negmr = pool.tile([128, 1], F32, name=f"ln_negmr_{tag}")
    nc.vector.tensor_scalar(out=negmr, in0=mv[:, 0:1], scalar1=rstd, scalar2=-1.0,
                            op0=OP.mult, op1=OP.mult)
    xs = pool.tile([128, DM], F32, name=f"ln_xs_{tag}")
    nc.scalar.activation(out=xs, in_=r, func=AF.Identity, scale=rstd, bias=negmr)
    return xs


# ---------------------------------------------------------------------------
# host side
# ---------------------------------------------------------------------------

_prog_cache = {}


def _get_program():
    if "nc" not in _prog_cache:
        _prog_cache["nc"] = build_program()
    return _prog_cache["nc"]


def _host_constants(ss, lsi):
    ss = np.asarray(ss, np.int64)
    lsi = np.asarray(lsi, np.int64)
    f = np.float32
    H = ss[:, 0].astype(np.int64)
    W = ss[:, 1].astype(np.int64)
    refd = np.zeros((LV, 2), f)
    refd[:, 0] = W; refd[:, 1] = H
    refdims = np.broadcast_to(refd.reshape(1, -1), (128, LV * 2)).copy()
    dm2 = np.zeros((M, LV, P, 2), f)
    dm2[:, :, :, 0] = (W - 2)[None, :, None]
    dm2[:, :, :, 1] = (H - 2)[None, :, None]
    dimm2 = np.broadcast_to(dm2.reshape(1, -1), (128, M * LV * P * 2)).copy()
    w8a = np.zeros((M, LV, P), f)
    w8a[:, :, :] = W[None, :, None]
    w8 = np.broadcast_to(w8a.reshape(1, -1), (128, M * LV * P)).copy()
    c0a = np.zeros((M, LV, P), f)
    c0a[:, :, :] = lsi[None, :, None]
    c0 = np.broadcast_to(c0a.reshape(1, -1), (128, M * LV * P)).copy()
    return refdims, dimm2, w8, c0


def _build_in_maps(inputs):
    src = np.asarray(inputs["src"], np.float32)
    q_feat = np.asarray(inputs["q_feat"], np.float32)
    q_pos = np.asarray(inputs["q_pos"], np.float32)
    ref = np.asarray(inputs["reference_points"], np.float32)
    ss = np.asarray(inputs["spatial_shapes"])
    lsi_in = np.asarray(inputs["level_start_index"])

    lsi = lsi_in.astype(np.int64)

    assert src.shape == (B, L, DM), src.shape
    refdims, dimm2, w8, c0 = _host_constants(ss, lsi)

    def as_bf16(a):
        import ml_dtypes
        return np.asarray(a, np.float32).astype(ml_dtypes.bfloat16)

    import ml_dtypes
    # LN1 affine fold: FFN consumes the normalized xs directly, so
    # W1 <- diag(ln1_g) @ W1, b1 <- b1 + ln1_b @ W1, b2 <- b2 + ln1_b.
    ln1_g = np.asarray(inputs["ln1_g"], np.float32)
    ln1_b = np.asarray(inputs["ln1_b"], np.float32)
    W1f = np.asarray(inputs["W1"], np.float32) * ln1_g[:, None]
    b1f = np.asarray(inputs["b1"], np.float32) + ln1_b @ np.asarray(inputs["W1"], np.float32)
    b2f = np.asarray(inputs["b2"], np.float32) + ln1_b
    common = {
        "Woff": as_bf16(inputs["W_off"]),
        "Wattn": as_bf16(inputs["W_attn"]),
        "Wv": as_bf16(inputs["W_v"]),
        "Wout": as_bf16(inputs["W_out"]),
        "W1": as_bf16(W1f),
        "W2": as_bf16(inputs["W2"]),
        "boff": as_bf16(inputs["b_off"]).reshape(1, -1),
        "battn": as_bf16(inputs["b_attn"]).reshape(1, -1),
        "bv": as_bf16(inputs["b_v"]).reshape(1, -1),
        "bout": as_bf16(inputs["b_out"]).reshape(1, -1),
        "b1": as_bf16(b1f).reshape(1, -1),
        "b2": as_bf16(b2f).reshape(1, -1),
        "ln1g": np.broadcast_to(np.asarray(inputs["ln1_g"], np.float32), (128, DM)).copy(),
        "ln1b": np.broadcast_to(np.asarray(inputs["ln1_b"], np.float32), (128, DM)).copy(),
        "ln2g": np.broadcast_to(np.asarray(inputs["ln2_g"], np.float32), (128, DM)).copy(),
        "ln2b": np.broadcast_to(np.asarray(inputs["ln2_b"], np.float32), (128, DM)).copy(),
        "ident": np.eye(128, dtype=np.float32).astype(ml_dtypes.bfloat16),
        "identf": np.eye(128, dtype=np.float32),
        "ones1": as_bf16(np.ones((1, 128), np.float32)),
        "refdims": refdims, "dimm2": dimm2, "w8": w8, "c0": c0,
    }

    query_full = q_feat + q_pos  # with_pos_embed precomputed host-side
    halves = [(0, LC), (LC, L - LC)]
    in_maps = []
    for core in range(8):
        b = core // 2
        h0, hn = halves[core % 2]
        qf = np.zeros((LCPAD, DM), np.float32)
        qp = np.zeros((LCPAD, DM), np.float32)
        rf = np.zeros((LCPAD, LV, 2), np.float32)
        qf[:hn] = q_feat[b, h0:h0 + hn]
        qp[:hn] = query_full[b, h0:h0 + hn]
        rf[:hn] = ref[b, h0:h0 + hn]
        m = dict(common)
        m.update({"qf": qf, "qp": qp, "ref": rf, "srcb": src[b]})
        in_maps.append(m)
    return in_maps


def kernel(**inputs):
    from concourse.bass_utils import run_bass_kernel_spmd

    in_maps = _build_in_maps(inputs)
    nc = _get_program()
    res = run_bass_kernel_spmd(nc, in_maps, core_ids=list(range(8)))

    halves = [(0, LC), (LC, L - LC)]
    out = np.zeros((B, L, DM), np.float32)
    for core in range(8):
        b = core // 2
        h0, hn = halves[core % 2]
        out[b, h0:h0 + hn] = res.results[core]["out"][:hn]
    return out
